# revision 1
# baseline (speedup 1.0000x reference)
"""Cy2MixerBlock (nn_Cy2MixerBlock_6700148982551) Trainium2 Bass kernel.

Per (b,t) slice (N=325 nodes, D=F=128 features), fully independent:
  natural  = (node partitions [3 tiles 128/128/69], feature free)
  f-major  = (feature partitions [exactly 128], node free [325])
Matmuls contract over partitions; LN stats reduce over free dim in natural
layout; transposes via PE identity-matmul; SGU conv slides over the FEATURE
axis -> implemented as 9 accumulated matmuls whose stationary operand is a
column-shifted view of a zero-padded vn tile.
"""

import numpy as np
import ml_dtypes
from contextlib import ExitStack

import concourse.bass as bass
import concourse.bacc as bacc
import concourse.mybir as mybir
import concourse.tile as tile
from concourse import bass_utils
from concourse.masks import make_identity

F32 = mybir.dt.float32
F32R = mybir.dt.float32r
BF16 = mybir.dt.bfloat16
AF = mybir.ActivationFunctionType
ALU = mybir.AluOpType

B, T, N, D = 32, 12, 325, 128
F = D
NCORES = 8
BPC = B // NCORES  # batches per core
PKS = (128, 128, 69)  # node-tile partition sizes
NP = 326  # node free dim padded even (fp32r matmul dst must be even-sized)
EPS = 1e-5


# ---------------------------------------------------------------- host folds
def host_fold(inp):
    """Exact algebraic folds (numpy fp32). Returns dict of device arrays +
    flags for the general (nonzero-bias) paths."""
    g = inp["norm_g"].astype(np.float32)
    bn = inp["norm_b"].astype(np.float32)
    Wqkv = inp["Wqkv"].astype(np.float32)
    bqkv = inp["bqkv"].astype(np.float32)
    # LN1 affine folded into all consumers of xn
    Wqkv_f = g[:, None] * Wqkv
    bqkv_f = bqkv + bn @ Wqkv
    W1e = inp["W1"].astype(np.float32) + inp["Wl"].astype(np.float32) @ inp["aff1_W"].astype(np.float32)
    b1e = inp["b1"].astype(np.float32) + inp["bl"].astype(np.float32) @ inp["aff1_W"].astype(np.float32) + inp["aff1_b"].astype(np.float32)
    W1f = g[:, None] * W1e
    b1f = b1e + bn @ W1e

    Wq = Wqkv_f[:, 0:F]
    Wk = Wqkv_f[:, F:2 * F]
    Wv = Wqkv_f[:, 2 * F:3 * F]
    A = (Wq @ Wk.T)  # logits = xn A xn^T
    bq = bqkv_f[0:F]
    bk = bqkv_f[F:2 * F]
    bv = bqkv_f[2 * F:3 * F]

    Wo = inp["Wo"].astype(np.float32)
    bo2 = inp["bo"].astype(np.float32) + bv @ Wo  # v-bias exact fold through softmax

    conv_w = inp["conv_w"].astype(np.float32)  # (N, N, 1, K)
    # conv rhs: wdtT[kw][k] = conv_w[:, k*128:+pk, 0, kw].T  -> (pk, 325), pad to 128
    wdtT = np.zeros((3, 3, 128, NP), np.float32)
    for kw in range(3):
        for k in range(3):
            pk = PKS[k]
            wdtT[kw, k, :pk, :N] = conv_w[:, k * 128:k * 128 + pk, 0, kw].T

    d = dict(
        A=A.astype(np.float32),
        wv=Wv.astype(np.float32),
        w1f=W1f.astype(np.float32),
        wo_bf=Wo.astype(ml_dtypes.bfloat16),
        w2=inp["W2"].astype(np.float32),
        wdtT=wdtT,
        cb=np.pad(inp["conv_b"].astype(np.float32), (0, NP - N))[None, :],
        ones_row=np.ones((1, 128), np.float32),
        ones_col_bf=np.ones((128, 1), ml_dtypes.bfloat16),
        zpad=np.zeros((128, 2), np.float32),
    )
    flags = dict(
        has_bqk=not (np.all(bq == 0) and np.all(bk == 0)),
        has_b1=not np.all(b1f == 0),
        has_bo2=not np.all(bo2 == 0),
        has_b2=not np.all(inp["b2"] == 0),
        has_sgu=not (np.all(inp["sgu_g"] == 1) and np.all(inp["sgu_b"] == 0)),
    )
    extras = dict(
        b1f=b1f, bo2=bo2, b2=inp["b2"].astype(np.float32),
        bq=bq, bk=bk, Wq=Wq, Wk=Wk,
        sgu_g=inp["sgu_g"].astype(np.float32), sgu_b=inp["sgu_b"].astype(np.float32),
    )
    return d, flags, extras


# ---------------------------------------------------------------- builder
def build_nc(flags, dbg=False, bpc=BPC, loop_n=None):
    nc = bacc.Bacc("TRN2", target_bir_lowering=False, debug=False)

    x_dram = nc.dram_tensor("x_sh", [bpc, T, N, D], F32, kind="ExternalInput")
    o_dram = nc.dram_tensor("out_sh", [bpc, T, N, D], F32, kind="ExternalOutput")
    wd = {}
    for nm, shp in [("A", [D, D]), ("wv", [D, F]), ("w1f", [D, 2 * F]), ("w2", [F, D])]:
        wd[nm] = nc.dram_tensor(nm, shp, F32R, kind="ExternalInput")
    wd["wo_bf"] = nc.dram_tensor("wo_bf", [F, F], BF16, kind="ExternalInput")
    wd["wdtT"] = nc.dram_tensor("wdtT", [3, 3, 128, NP], F32R, kind="ExternalInput")
    wd["cb"] = nc.dram_tensor("cb", [1, NP], F32R, kind="ExternalInput")
    wd["ones_row"] = nc.dram_tensor("ones_row", [1, 128], F32R, kind="ExternalInput")
    wd["ones_col_bf"] = nc.dram_tensor("ones_col_bf", [128, 1], BF16, kind="ExternalInput")
    wd["zpad"] = nc.dram_tensor("zpad", [128, 2], F32R, kind="ExternalInput")
    if flags["has_b1"]:
        wd["b1fT"] = nc.dram_tensor("b1fT", [2 * F, 1], F32, kind="ExternalInput")
        wd["b1vB"] = nc.dram_tensor("b1vB", [128, F], F32, kind="ExternalInput")
    if flags["has_bo2"]:
        wd["bo2T"] = nc.dram_tensor("bo2T", [F, 1], F32, kind="ExternalInput")
    if flags["has_b2"]:
        wd["b2T"] = nc.dram_tensor("b2T", [D, 1], F32, kind="ExternalInput")
    if flags["has_sgu"]:
        wd["gsB"] = nc.dram_tensor("gsB", [128, F], F32, kind="ExternalInput")
        wd["bsB"] = nc.dram_tensor("bsB", [128, F], F32, kind="ExternalInput")
    assert not flags["has_bqk"], "bq/bk general path not implemented"

    dbg_outs = {}
    if dbg:
        for nm, shp in [("d_xnt", [T, D, NP]), ("d_att", [T, F, NP]),
                        ("d_u", [T, F, NP]), ("d_vn", [T, 128, 3, 130]),
                        ("d_sg", [T, F, NP])]:
            dbg_outs[nm] = nc.dram_tensor(nm, [bpc] + shp, F32, kind="ExternalOutput")

    x_ap = x_dram.ap()
    o_ap = o_dram.ap()

    with tile.TileContext(nc) as tc:
        with ExitStack() as ctx:
            const = ctx.enter_context(tc.tile_pool(name="const", bufs=1))
            wrk = ctx.enter_context(tc.tile_pool(name="wrk", bufs=3))
            wrk3 = ctx.enter_context(tc.tile_pool(name="wrk3", bufs=4))
            ps8 = ctx.enter_context(tc.tile_pool(name="ps8", bufs=8, space="PSUM"))

            # ---- constants
            ident = const.tile([128, 128], F32, name="ident")
            make_identity(nc, ident)
            zero_sb = const.tile([128, 1], F32, name="zero_sb")
            nc.vector.memset(zero_sb, 0.0)
            csb = {}
            for nm, t_ in wd.items():
                if nm == "wdtT":
                    csb[nm] = const.tile([128, 3, 3, NP], t_.dtype, name="c_wdtT")
                    nc.sync.dma_start(out=csb[nm], in_=t_.ap().rearrange("a b p n -> p a b n"))
                    continue
                shp = list(t_.shape)
                csb[nm] = const.tile(shp, t_.dtype, name=f"c_{nm}")
                nc.sync.dma_start(out=csb[nm], in_=t_.ap())

            xnt_bufs = []
            for bi in range(6):
                xb = const.tile([128, NP], F32R, name=f"xntbuf{bi}")
                nc.sync.dma_start(out=xb[:, N:NP], in_=csb["zpad"][:, 0:1])
                xnt_bufs.append(xb)

            vnp_bufs = []
            for bi in range(3):
                vb = const.tile([128, 3, 130], F32R, name=f"vnpbuf{bi}")
                if dbg:
                    nc.vector.memset(vb.bitcast(F32), 0.0)
                for k in range(3):
                    nc.sync.dma_start(out=vb[:, k, 0:130:129], in_=csb["zpad"])
                vnp_bufs.append(vb)

            A_sb, wv_sb, w1f_sb, w2_sb = csb["A"], csb["wv"], csb["w1f"], csb["w2"]
            wo_sb, wdtT_sb, cb_sb = csb["wo_bf"], csb["wdtT"], csb["cb"]
            ones_row, ones_col = csb["ones_row"], csb["ones_col_bf"]

            def ts_(k):
                return slice(k * 128, k * 128 + PKS[k])

            MAGIC = 0x5F3759DF

            def rsqrt_cols(pool, var_ap, n):
                """rstd = (var+EPS)^-1/2 on DVE only (bit-trick seed + 2 Newton
                iters, exact to fp32 ulp). var_ap: (128, n) strided view."""
                t_ = pool.tile([128, 4, 3], F32, tag="nw", name="nw")
                t = t_[:, 0, 0:n]
                y = t_[:, 1, 0:n]
                a = t_[:, 2, 0:n]
                c = t_[:, 3, 0:n]
                nc.vector.tensor_scalar_add(t, var_ap, EPS)
                nc.vector.tensor_scalar(
                    out=y.bitcast(mybir.dt.int32), in0=t.bitcast(mybir.dt.int32),
                    scalar1=1, scalar2=None, op0=ALU.logical_shift_right)
                nc.vector.tensor_scalar(
                    out=y.bitcast(mybir.dt.int32), in0=y.bitcast(mybir.dt.int32),
                    scalar1=-1, scalar2=MAGIC, op0=ALU.mult, op1=ALU.add)
                for _ in range(1):
                    nc.vector.tensor_mul(a, y, y)
                    nc.vector.tensor_mul(a, a, t)
                    nc.vector.tensor_scalar(out=c, in0=a, scalar1=-0.5, scalar2=1.5,
                                            op0=ALU.mult, op1=ALU.add)
                    nc.vector.tensor_mul(y, y, c)
                return y

            MAGIC = 0x5F3759DF

            def rsqrt_cols(pool, var_ap, n, eng=None):
                """rstd = (var+EPS)^-1/2, bit-trick seed + 1 Newton iter
                (rel err ~3e-6), on DVE or GPSIMD."""
                e = eng or nc.vector
                t_ = pool.tile([128, 4, 3], F32, tag="nw", name="nw")
                t = t_[:, 0, 0:n]
                y = t_[:, 1, 0:n]
                a = t_[:, 2, 0:n]
                c = t_[:, 3, 0:n]
                e.tensor_scalar_add(t, var_ap, EPS)
                e.tensor_scalar(
                    out=y.bitcast(mybir.dt.int32), in0=t.bitcast(mybir.dt.int32),
                    scalar1=1, scalar2=None, op0=ALU.logical_shift_right)
                e.tensor_scalar(
                    out=y.bitcast(mybir.dt.int32), in0=y.bitcast(mybir.dt.int32),
                    scalar1=-1, scalar2=MAGIC, op0=ALU.mult, op1=ALU.add)
                for _ in range(1):
                    e.tensor_mul(a, y, y)
                    e.tensor_mul(a, a, t)
                    e.tensor_scalar(out=c, in0=a, scalar1=-0.5, scalar2=1.5,
                                    op0=ALU.mult, op1=ALU.add)
                    e.tensor_mul(y, y, c)
                return y

            def s1(b, t):
                st_ = {}
                S[(b, t)] = st_
# ---------- load x natural (3 node tiles)
                xt = wrk3.tile([128, 3, 128], F32, tag="xt", bufs=6, name="xt")
                nc.sync.dma_start(
                    out=xt[:, 0:2, :],
                    in_=x_ap[b, t, 0:256, :].rearrange("(k p) d -> p k d", p=128))
                nc.sync.dma_start(out=xt[:69, 2, :], in_=x_ap[b, t, 256:325, :])

                # ---------- LN1 (natural): batched stats + DVE rsqrt
                mv1 = wrk3.tile([128, 3, 2], F32, tag="mv1", name="mv1")
                nc.vector.memset(mv1[64:128, 2, :], 1.0)
                for k in range(3):
                    pk = PKS[k]
                    stx = wrk3.tile([128, 6], F32, tag="st", name="stx")
                    nc.vector.bn_stats(stx[:pk], xt[:pk, k, :])
                    nc.vector.bn_aggr(mv1[:pk, k, :], stx[:pk])
                rs1 = rsqrt_cols(wrk3, mv1[:, :, 1], 3)
                XnT = xnt_bufs[(b * T + t) % 6]
                tpx = ps8.tile([128, 3, 128], F32, tag="ps", name="tpx")
                for k in range(3):
                    pk = PKS[k]
                    xtmp = wrk3.tile([128, 128], F32, tag="xtmp", name="xtmp")
                    nc.gpsimd.tensor_scalar(
                        out=xtmp[:pk], in0=xt[:pk, k, :], scalar1=mv1[:pk, k, 0:1],
                        scalar2=rs1[:pk, k:k + 1], op0=ALU.subtract, op1=ALU.mult)
                    nc.tensor.transpose(tpx[:, k, :pk], xtmp[:pk, :], ident[:pk, :pk])
                nc.scalar.copy(out=XnT[:, 0:256], in_=tpx[:, 0:2, :])
                nc.scalar.copy(out=XnT[:, 256:325], in_=tpx[:, 2, :69])
                if dbg:
                    nc.sync.dma_start(out=dbg_outs["d_xnt"].ap()[b, t], in_=XnT.bitcast(F32))
                st_["xt"] = xt
                st_["XnT"] = XnT

            def s2a(b, t):
                st_ = S[(b, t)]
                XnT = st_["XnT"]
                Xtr = XnT
# ---------- tiny attention
                g_ps = ps8.tile([128, NP], F32, tag="ps", name="g_ps")
                nc.tensor.matmul(g_ps, A_sb, Xtr, start=True, stop=True)
                G = wrk.tile([128, NP], F32R, tag="G", name="G")
                nc.scalar.copy(G, g_ps)
                E = wrk.tile([128, 3, NP], BF16, tag="E", bufs=3, name="E")
                for k in range(3):
                    pk = PKS[k]
                    lt = ps8.tile([128, NP], F32, tag="ps", name="lt")
                    nc.tensor.matmul(lt[:pk], XnT[:, ts_(k)],
                                     G, start=True, stop=True)
                    nc.scalar.activation(E[:pk, k, :], lt[:pk], AF.Exp)
                st_["E"] = E

            def s2a2(b, t):
                st_ = S[(b, t)]
                XnT = st_["XnT"]
                E = st_["E"]
                z_ps = ps8.tile([1, NP], F32, tag="ps", name="z_ps")
                for k in range(3):
                    pk = PKS[k]
                    nc.tensor.matmul(z_ps, ones_col[:pk], E[:pk, k, :],
                                     start=(k == 0), stop=(k == 2))
                zr = wrk.tile([1, NP], F32R, tag="zr", name="zr")
                with nc.allow_low_precision(reason="f32r rounding of 1/Z is fine"):
                    nc.vector.reciprocal(zr, z_ps)
                zb_ps = ps8.tile([128, NP], F32, tag="ps", name="zb_ps")
                nc.tensor.matmul(zb_ps, ones_row, zr, start=True, stop=True)
                zb = wrk.tile([128, NP], F32, tag="zb", name="zb")
                nc.scalar.copy(zb, zb_ps)
                vns = wrk.tile([128, 3, 128], BF16, tag="vns", name="vns")
                vp = ps8.tile([128, 3, 128], F32, tag="ps", name="vp")
                for k in range(3):
                    pk = PKS[k]
                    nc.tensor.matmul(vp[:pk, k, :], XnT[:, ts_(k)],
                                     wv_sb, start=True, stop=True)
                    nc.scalar.copy(out=vns[:pk, k, :], in_=vp[:pk, k, :])
                au = ps8.tile([128, NP], F32, tag="ps", name="au")
                for k in range(3):
                    pk = PKS[k]
                    nc.tensor.matmul(au, vns[:pk, k, :], E[:pk, k, :],
                                     start=(k == 0), stop=(k == 2))
                att = wrk.tile([128, NP], BF16, tag="att", bufs=3, name="att")
                nc.vector.tensor_mul(att, au, zb)
                if dbg:
                    dat = wrk.tile([128, NP], F32, tag="dat", name="dat")
                    nc.vector.tensor_copy(dat, att)
                    nc.sync.dma_start(out=dbg_outs["d_att"].ap()[b, t], in_=dat)
                st_["att"] = att

            def s2b(b, t):
                st_ = S[(b, t)]
                XnT = st_["XnT"]
                Xtr = XnT
# ---------- u (f-major) and vv -> LN2 -> vn_pad (natural)
                up = ps8.tile([128, NP], F32, tag="ps", name="up")
                nc.tensor.matmul(up, w1f_sb[:, 0:F], Xtr,
                                 start=True, stop=True)
                u = wrk.tile([128, NP], F32, tag="u", bufs=3, name="u")
                ub = csb["b1fT"][0:F] if flags["has_b1"] else zero_sb
                nc.scalar.activation(u, up, AF.Relu, bias=ub)
                if dbg:
                    nc.sync.dma_start(out=dbg_outs["d_u"].ap()[b, t], in_=u)

                vn_pad = vnp_bufs[(b * T + t) % 3]
                mv2 = wrk3.tile([128, 3, 2], F32, tag="mv1", name="mv2")
                nc.vector.memset(mv2[64:128, 2, :], 1.0)
                vvn = wrk3.tile([128, 3, 128], F32, tag="vvn", name="vvn")
                vvp = ps8.tile([128, 3, 128], F32, tag="ps", name="vvp")
                for k in range(3):
                    pk = PKS[k]
                    nc.tensor.matmul(vvp[:pk, k, :], XnT[:, ts_(k)],
                                     w1f_sb[:, F:2 * F],
                                     start=True, stop=True)
                for k in range(3):
                    pk = PKS[k]
                    if flags["has_b1"]:
                        nc.vector.tensor_add(vvn[:pk, k, :], vvp[:pk, k, :], csb["b1vB"][:pk])
                        nc.vector.tensor_scalar_max(vvn[:pk, k, :], vvn[:pk, k, :], 0.0)
                    else:
                        if k < 2:
                            continue
                        nc.vector.tensor_scalar_max(vvn[:, 0:2, :], vvp[:, 0:2, :], 0.0)
                        nc.vector.tensor_scalar_max(vvn[:69, 2, :], vvp[:69, 2, :], 0.0)
                for k in range(3):
                    pk = PKS[k]
                    st2 = wrk3.tile([128, 6], F32, tag="st", name="st2")
                    nc.vector.bn_stats(st2[:pk], vvn[:pk, k, :])
                    nc.vector.bn_aggr(mv2[:pk, k, :], st2[:pk])
                rs2 = rsqrt_cols(wrk3, mv2[:, :, 1], 3)
                for k in range(3):
                    pk = PKS[k]
                    nc.gpsimd.tensor_scalar(
                        out=vn_pad[:pk, k, 1:129], in0=vvn[:pk, k, :],
                        scalar1=mv2[:pk, k, 0:1], scalar2=rs2[:pk, k:k + 1],
                        op0=ALU.subtract, op1=ALU.mult)
                    if flags["has_sgu"]:
                        nc.vector.tensor_mul(vn_pad[:pk, k, 1:129], vn_pad[:pk, k, 1:129], csb["gsB"][:pk])
                        nc.vector.tensor_add(vn_pad[:pk, k, 1:129], vn_pad[:pk, k, 1:129], csb["bsB"][:pk])
                if dbg:
                    nc.sync.dma_start(out=dbg_outs["d_vn"].ap()[b, t], in_=vn_pad.bitcast(F32))
                st_["u"] = u
                st_["vn_pad"] = vn_pad

            def s3(b, t):
                st_ = S[(b, t)]
                xt = st_["xt"]
                att = st_["att"]
                u = st_["u"]
                vn_pad = st_["vn_pad"]
# ---------- conv over f (9 MMs) + conv bias rank-1 + gate
                co = ps8.tile([128, NP], F32, tag="ps", name="co")
                first = True
                for kw in range(3):
                    for k in range(3):
                        pk = PKS[k]
                        nc.tensor.matmul(co, vn_pad[:pk, k, kw:kw + 128],
                                         wdtT_sb[:pk, kw, k, :],
                                         start=first, stop=False)
                        first = False
                nc.tensor.matmul(co, ones_row, cb_sb,
                                 start=False, stop=False)
                nc.tensor.matmul(co, wo_sb, att, start=False, stop=True)
                # ---------- sg = (co + bo2) * u ; out = relu(W2^T sg) ; +residual
                sg = wrk.tile([128, NP], F32R, tag="sg", name="sg")
                bo_s = csb["bo2T"] if flags["has_bo2"] else 0.0
                nc.vector.scalar_tensor_tensor(out=sg, in0=co, scalar=bo_s, in1=u,
                                               op0=ALU.add, op1=ALU.mult)
                if dbg:
                    nc.sync.dma_start(out=dbg_outs["d_sg"].ap()[b, t], in_=sg.bitcast(F32))
                op_ = ps8.tile([128, NP], F32, tag="ps", name="op_")
                nc.tensor.matmul(op_, w2_sb, sg,
                                 start=True, stop=True)
                ot = wrk.tile([128, NP], F32, tag="ot", name="ot")
                b2b = csb["b2T"] if flags["has_b2"] else zero_sb
                nc.scalar.activation(ot, op_, AF.Relu, bias=b2b)
                onat = wrk.tile([128, 3, 128], F32, tag="onat", name="onat")
                tpo = ps8.tile([128, 3, 128], F32, tag="ps", name="tpo")
                for k in range(3):
                    pk = PKS[k]
                    nc.tensor.transpose(tpo[:pk, k, :], ot[:, ts_(k)], ident)
                nc.vector.tensor_add(onat[:, 0:2, :], tpo[:, 0:2, :], xt[:, 0:2, :])
                nc.vector.tensor_add(onat[:69, 2, :], tpo[:69, 2, :], xt[:69, 2, :])
                nc.scalar.dma_start(
                    out=o_ap[b, t, 0:256, :].rearrange("(k p) d -> p k d", p=128),
                    in_=onat[:, 0:2, :])
                nc.scalar.dma_start(out=o_ap[b, t, 256:325, :], in_=onat[:69, 2, :])

            seq = [(b, t) for b in range(bpc) for t in range(T)]

            def emit_all():
                S.clear()
                for i in range(len(seq) + 4):
                    if i < len(seq):
                        s1(*seq[i])
                    if 1 <= i < len(seq) + 1:
                        s2a(*seq[i - 1])
                    if 2 <= i < len(seq) + 2:
                        s2a2(*seq[i - 2])
                    if 3 <= i < len(seq) + 3:
                        s2b(*seq[i - 3])
                    if 4 <= i < len(seq) + 4:
                        s3(*seq[i - 4])
                        del S[seq[i - 4]]

            S = {}
            if loop_n:
                with tc.For_i(0, loop_n, 1):
                    emit_all()
            else:
                emit_all()

    nc.compile()
    return nc


# ---------------------------------------------------------------- runner
def make_in_maps(inputs, dev, flags, extras):
    x = np.ascontiguousarray(inputs["x"], dtype=np.float32)
    maps = []
    for c in range(NCORES):
        m = dict(dev)
        if flags["has_b1"]:
            m["b1fT"] = extras["b1f"][:, None].astype(np.float32)
            m["b1vB"] = np.broadcast_to(extras["b1f"][None, F:2 * F], (128, F)).astype(np.float32).copy()
        if flags["has_bo2"]:
            m["bo2T"] = extras["bo2"][:, None].astype(np.float32)
        if flags["has_b2"]:
            m["b2T"] = extras["b2"][:, None].astype(np.float32)
        if flags["has_sgu"]:
            m["gsB"] = np.broadcast_to(extras["sgu_g"][None, :], (128, F)).astype(np.float32).copy()
            m["bsB"] = np.broadcast_to(extras["sgu_b"][None, :], (128, F)).astype(np.float32).copy()
        m["x_sh"] = x[c * BPC:(c + 1) * BPC]
        maps.append(m)
    return maps


_NC_CACHE = {}


def kernel(**inputs):
    dev, flags, extras = host_fold(inputs)
    key = tuple(sorted(flags.items()))
    if key not in _NC_CACHE:
        _NC_CACHE[key] = build_nc(flags, dbg=False)
    nc = _NC_CACHE[key]
    in_maps = make_in_maps(inputs, dev, flags, extras)
    res = bass_utils.run_bass_kernel_spmd(nc, in_maps, core_ids=list(range(NCORES)))
    out = np.concatenate([res.results[c]["out_sh"] for c in range(NCORES)], axis=0)
    return np.ascontiguousarray(out, dtype=np.float32)

